# revision 1
# baseline (speedup 1.0000x reference)
"""SSIM(3x3 avg-pool) + L1 loss kernel for Trainium2, 8 NeuronCores.

loss = 0.85 * mean(clip((1 - ssim_map)/2, 0, 1)) + 0.15 * mean(|pred - target|)

Full inputs pred/target: (16, 1, 1024, 1024) f32. Data-parallel: 2 images per
core; each core returns per-partition partial sums [128, 2] (col 0 = sum of
the clipped ssim loss map, col 1 = sum |pred-target|); the host combines and
applies the means / alpha-beta weights.

Math (per image pair, variance identities halve the pooled field count):
  u = p + t, v = p - t
  box(x) = 3x3 zero-padded box sum / 9 (separable)
  X = box(p), Y = box(t), G = box(u^2), Hh = box(v^2)
  n1*n2 = (2XY + C1) * ((G-Hh)/2 - 2XY + C2)
  d1*d2 = (X^2+Y^2 + C1) * ((G+Hh)/2 - (X^2+Y^2) + C2)
  ssim_map = n1*n2/(d1*d2);  contrib = clip(0.5 - 0.5*ssim_map, 0, 1)
  l1 from |v|.

Layout: the host packs each image pair row-wise as [0 p0 0|0 p1 0|0 t0 0|0 t1 0]
(zero-padded sections of width W+2), so every pre-pool stage (horizontal 3-tap
via shifted adds, u/v, squares, |v|) is ONE wide instruction across both
images and both tensors. The vertical 3-tap runs on the TensorEngine as a
banded [128 -> <=126] float32r matmul per 512-col chunk into a single 8-bank
PSUM tile; row halos come from overlapped 128-row stripes, image edges from
per-block banded matrices. Post-pool math uses custom fused DVE ops
(x^2+y^2, the (a+c0)(b*c1-a+c2) rational terms, and a fused
clip-and-accumulate), one reciprocal_approx_fast for the division.
"""

import sys

import numpy as np

sys.path.insert(0, "/opt/trn_rl_repo")

ALPHA = 0.85
BETA = 0.15
C1 = 0.01 ** 2
C2 = 0.03 ** 2

N_CORES = 8
IMG_H = 1024
IMG_W = 1024
N_IMG_PER_CORE = 2
BLK = 126          # output rows per vertical-matmul block
MAXW_PSUM = 512    # fp32 columns per PSUM bank

MM_F32R = True     # float32r matmuls: 4x PE throughput, ~1e-6 rel error
MM_BF16 = False    # bf16 h3 boxes bias the clipped-mean by ~4% - keep f32r

# --- custom fused DVE ops (registered into concourse.dve_ops at build) ---- #
_OP_SQSUM = None       # out = in0^2 + in1^2
_OP_SSIM_RAT = None    # out = (in0 + s0) * (in1*s1 - in0 + imm2)
_OP_SSIM_FINAL = None  # out = (s0 - clamp(in0*in1, s1, s0))*imm2; accum += out
_CUSTOM_OPS_OK = False


def _register_custom_ops():
    global _OP_SQSUM, _OP_SSIM_RAT, _OP_SSIM_FINAL, _CUSTOM_OPS_OK
    if _CUSTOM_OPS_OK:
        return
    from operator import add

    import concourse.dve_ops as dv
    from concourse.dve_spec import (
        C0, C1 as SC1, C2 as SC2, Spec, Src0, Src1, Zero, lower, maxx, minn, sq,
    )
    from concourse.dve_uop import DveOpSpec

    def _sqsum_ref(in0, in1, c0, c1, c2):
        return in0.astype(np.float32) ** 2 + in1.astype(np.float32) ** 2

    def _rat_ref(in0, in1, c0, c1, c2):
        a = in0.astype(np.float32)
        return (a + c0) * (in1.astype(np.float32) * c1 - a + c2)

    def _final_ref(in0, in1, c0, c1, c2):
        z = in0.astype(np.float32) * in1.astype(np.float32)
        b = ((c0 - np.clip(z, c1, c0)) * c2).astype(np.float32)
        return b, b.reshape(b.shape[0], -1).sum(axis=-1, keepdims=True)

    defs = [
        ("SSIM_SQSUM_ANT", Spec(body=sq(Src0) + sq(Src1), reference=_sqsum_ref)),
        ("SSIM_RAT_ANT", Spec(
            body=(Src0 + C0) * (Src1 * SC1 - Src0 + SC2), reference=_rat_ref)),
        ("SSIM_FINAL_ANT", Spec(
            body=(C0 - maxx(minn(Src0 * Src1, C0), SC1)) * SC2,
            accum=add, accum_init=Zero, reference=_final_ref)),
    ]
    made = {}
    for name, spec in defs:
        if name not in dv._SUB_OPCODE_FOR_NAME:
            stub = dv.DveOp(name, spec, subdim=False, uops_sha={})
            dv.OPS.append(stub)
            dv._SUB_OPCODE_FOR_NAME[name] = (
                dv._CUSTOM_DVE_ROW_BASE + len(dv.OPS) - 1
            )
            dv.CUSTOM_DVE_SPECS[name] = spec
        opcode = dv._SUB_OPCODE_FOR_NAME[name]
        shas = {}
        for ver in ("v3", "v4"):
            res = DveOpSpec(
                name=name, opcode=opcode, uops=lower(spec, ver=ver),
                rd1_en=dv.has_src1(spec),
            )
            shas[ver] = res.sha(ver)
        op = dv.DveOp(name, spec, subdim=False, uops_sha=shas)
        idx = next(i for i, o in enumerate(dv.OPS) if o.name == name)
        dv.OPS[idx] = op
        dv.CUSTOM_DVE_SPECS[name] = spec
        made[name] = op
    _OP_SQSUM = made["SSIM_SQSUM_ANT"]
    _OP_SSIM_RAT = made["SSIM_RAT_ANT"]
    _OP_SSIM_FINAL = made["SSIM_FINAL_ANT"]
    _CUSTOM_OPS_OK = True


def _blocks(H):
    """Vertical block decomposition: list of (r0, n_out, rs, nr)."""
    out = []
    b = 0
    while b * BLK < H:
        r0 = b * BLK
        n_out = min(BLK, H - r0)
        rs = max(r0 - 1, 0)
        re = min(r0 + n_out, H - 1)
        out.append((r0, n_out, rs, re - rs + 1))
        b += 1
    return out


def make_bmats(H):
    """Banded vertical-sum matrices, padded into [nblk,128,BLK]. Entries are
    1/9 (normalized boxes) or 1.0 when MM_BF16 (bf16-exact; the /9 moves into
    81-scaled ssim constants)."""
    blocks = _blocks(H)
    bm = np.zeros((len(blocks), 128, BLK), dtype=np.float32)
    ninth = np.float32(1.0) if MM_BF16 else np.float32(1.0) / np.float32(9.0)
    for i, (r0, n_out, rs, nr) in enumerate(blocks):
        for k in range(nr):
            for j in range(n_out):
                if abs((rs + k) - (r0 + j)) <= 1:
                    bm[i, k, j] = ninth
    return bm


def build_program(n_img, H, W, io_internal=False):
    """Build the per-core program for n_img (even) HxW images.

    DRAM input "ptin": [ (n_img/2)*H, 4*(W+2) ] with row layout
    [0 p0 0 | 0 p1 0 | 0 t0 0 | 0 t1 0] per image pair.
    io_internal makes ptin Internal DRAM (timing-only builds).
    """
    import concourse.bacc as bacc
    import concourse.tile as tile
    from concourse import mybir

    assert n_img % 2 == 0
    f32 = mybir.dt.float32
    Alu = mybir.AluOpType
    Act = mybir.ActivationFunctionType

    blocks = _blocks(H)
    nblk = len(blocks)
    S = W + 2                       # one padded section
    S4 = 4 * S                      # packed row width
    npairs = n_img // 2
    n_chunks = (W + MAXW_PSUM - 1) // MAXW_PSUM
    W2 = 2 * W                      # field-pair width (img0|img1)

    _register_custom_ops()
    nc = bacc.Bacc("TRN2", target_bir_lowering=False, debug=False)

    io_kind = "Internal" if io_internal else "ExternalInput"
    ptin_d = nc.dram_tensor("ptin", [npairs * H, S4], f32, kind=io_kind).ap()
    bm_d = nc.dram_tensor("bmats", [nblk, 128, BLK], f32, kind="ExternalInput").ap()
    acc_d = nc.dram_tensor("acc_out", [128, 2], f32, kind="ExternalOutput").ap()

    with tile.TileContext(nc) as tc:
        with (
            tc.tile_pool(name="consts", bufs=1) as cpool,
            tc.tile_pool(name="io", bufs=2) as iopool,
            tc.tile_pool(name="hsum", bufs=2) as hpool,
            tc.tile_pool(name="post", bufs=1) as ppool,
            tc.tile_pool(name="psum", bufs=1, space="PSUM") as psumpool,
        ):
            acc = cpool.tile([128, 2], f32, tag="acc")
            nc.vector.memset(acc[:, :], 0.0)
            if io_internal:
                fill = cpool.tile([128, S4], f32, tag="fill")
                nc.vector.memset(fill[:, :], 0.625)
                rows_total = npairs * H
                for r in range(0, rows_total, 128):
                    nrr = min(128, rows_total - r)
                    nc.sync.dma_start(out=ptin_d[r:r + nrr, :], in_=fill[0:nrr, :])

            mm_dt = (mybir.dt.bfloat16 if MM_BF16
                     else (mybir.dt.float32r if MM_F32R else f32))
            mm_n = MAXW_PSUM
            n_chunks_mm = (W + mm_n - 1) // mm_n
            cC1 = float(C1) * (81.0 if MM_BF16 else 1.0)
            cC2 = float(C2) * (81.0 if MM_BF16 else 1.0)
            bmats = []
            for i, (r0, n_out, rs, nr) in enumerate(blocks):
                braw = cpool.tile([128, BLK], f32, tag=f"bmraw{i}", name="braw")
                nc.sync.dma_start(out=braw[0:nr, 0:n_out], in_=bm_d[i, 0:nr, 0:n_out])
                if MM_F32R or MM_BF16:
                    bt = cpool.tile([128, BLK], mm_dt, tag=f"bmat{i}", name="bt")
                    nc.vector.tensor_copy(bt[0:nr, 0:n_out], braw[0:nr, 0:n_out])
                else:
                    bt = braw
                bmats.append(bt)

            for pair in range(npairs):
                base = pair * H
                for bi, (r0, n_out, rs, nr) in enumerate(blocks):
                    # rows [0:k_l1] of consecutive stripes tile H exactly once
                    if bi + 1 < len(blocks):
                        k_l1 = blocks[bi + 1][2] - rs
                    else:
                        k_l1 = nr

                    pt = iopool.tile([128, S4], f32, tag="pt")
                    nc.sync.dma_start(
                        out=pt[0:nr, :], in_=ptin_d[base + rs: base + rs + nr, :])

                    rows = slice(0, nr)
                    # horizontal 3-tap for p0,p1,t0,t1 in two ops
                    # (junk at section tails is never read)
                    g = hpool.tile([128, S4 - 1], f32, tag="g")
                    nc.vector.tensor_add(
                        g[rows, :], pt[rows, 0:S4 - 1], pt[rows, 1:S4])
                    h3pt = hpool.tile([128, S4 - 2], mm_dt, tag="h3pt")
                    nc.vector.tensor_add(
                        h3pt[rows, :], g[rows, 0:S4 - 2], pt[rows, 2:S4])

                    # in place: t-half <- v = p - t ; p-half <- u = 2p - v
                    nc.gpsimd.tensor_sub(
                        pt[rows, 2 * S:S4], pt[rows, 0:2 * S], pt[rows, 2 * S:S4])
                    nc.vector.scalar_tensor_tensor(
                        pt[rows, 0:2 * S], pt[rows, 0:2 * S], 2.0,
                        pt[rows, 2 * S:S4], op0=Alu.mult, op1=Alu.subtract)
                    # L1 partial: |v| in place over the disjoint-cover rows
                    l1part = ppool.tile([128, 1], f32, tag="l1part")
                    nc.scalar.activation(
                        pt[0:k_l1, 2 * S:S4], pt[0:k_l1, 2 * S:S4], Act.Abs,
                        accum_out=l1part[0:k_l1, :])
                    # squares in place: [u0 u1 v0 v1] -> [u0^2 u1^2 v0^2 v1^2]
                    nc.scalar.activation(pt[rows, :], pt[rows, :], Act.Square)

                    g2 = hpool.tile([128, S4 - 1], f32, tag="g", name="g2")
                    nc.vector.tensor_add(
                        g2[rows, :], pt[rows, 0:S4 - 1], pt[rows, 1:S4])
                    h3uv = hpool.tile([128, S4 - 2], mm_dt, tag="h3uv")
                    nc.vector.tensor_add(
                        h3uv[rows, :], g2[rows, 0:S4 - 2], pt[rows, 2:S4])

                    bmat = bmats[bi]
                    ro = slice(0, n_out)
                    pw = slice(0, W2)

                    def mm_group(h3, ps):
                        # fields [f0_img0|f0_img1|f1_img0|f1_img1] -> PSUM
                        for s in range(4):
                            for ci in range(n_chunks_mm):
                                c0 = ci * mm_n
                                cw = min(mm_n, W - c0)
                                nc.tensor.matmul(
                                    ps[0:n_out, s * W + c0: s * W + c0 + cw],
                                    lhsT=bmat[0:nr, 0:n_out],
                                    rhs=h3[0:nr, s * S + c0: s * S + c0 + cw],
                                    start=True, stop=True)

                    # group 1: X|Y
                    ps = psumpool.tile([128, 4 * W], f32, tag="ps", name="ps")
                    mm_group(h3pt, ps)
                    Ysb = ppool.tile([128, W2], f32, tag="Ysb", name="Ysb")
                    nc.scalar.copy(Ysb[ro, :], ps[ro, W2:4 * W])
                    A2 = ppool.tile([128, W2], f32, tag="A2", name="A2")
                    nc.vector.scalar_tensor_tensor(
                        A2[ro, pw], ps[ro, 0:W2], 2.0, Ysb[ro, pw],
                        op0=Alu.mult, op1=Alu.mult)
                    V = ppool.tile([128, W2], f32, tag="V", name="V")
                    nc.vector._custom_dve(
                        _OP_SQSUM, out=V[ro, pw], in0=ps[ro, 0:W2], in1=Ysb[ro, pw])

                    # group 2: G|Hh (reuses the PSUM banks)
                    ps2 = psumpool.tile([128, 4 * W], f32, tag="ps", name="ps2")
                    mm_group(h3uv, ps2)
                    Hsb = ppool.tile([128, W2], f32, tag="Hsb", name="Hsb")
                    nc.scalar.copy(Hsb[ro, :], ps2[ro, W2:4 * W])
                    Dd = ppool.tile([128, W2], f32, tag="Dd", name="Dd")
                    nc.vector.tensor_sub(Dd[ro, pw], ps2[ro, 0:W2], Hsb[ro, pw])
                    M = ppool.tile([128, W2], f32, tag="M", name="M")
                    nc.vector.tensor_add(M[ro, pw], ps2[ro, 0:W2], Hsb[ro, pw])

                    # in-place: n1n2 -> A2's tile, d1d2 -> V, rcp -> M, fin -> Dd
                    n1n2 = A2
                    nc.vector._custom_dve(
                        _OP_SSIM_RAT, out=n1n2[ro, pw], in0=A2[ro, pw],
                        in1=Dd[ro, pw], s0=cC1, s1=0.5, imm2=cC2)
                    d1d2 = V
                    nc.vector._custom_dve(
                        _OP_SSIM_RAT, out=d1d2[ro, pw], in0=V[ro, pw],
                        in1=M[ro, pw], s0=cC1, s1=0.5, imm2=cC2)
                    rcp = M
                    nc.vector.reciprocal_approx_fast(rcp[ro, pw], d1d2[ro, pw])
                    fin = Dd
                    spart = ppool.tile([128, 1], f32, tag="spart")
                    nc.vector._custom_dve(
                        _OP_SSIM_FINAL, out=fin[ro, pw], in0=n1n2[ro, pw],
                        in1=rcp[ro, pw], s0=1.0, s1=-1.0, imm2=0.5,
                        accum_out=spart[ro, :])
                    nc.vector.tensor_add(
                        acc[0:n_out, 0:1], acc[0:n_out, 0:1], spart[ro, :])
                    nc.vector.tensor_add(
                        acc[0:k_l1, 1:2], acc[0:k_l1, 1:2], l1part[0:k_l1, :])

            nc.sync.dma_start(out=acc_d[:, :], in_=acc[:, :])

    nc.compile()
    return nc


_CACHE = {}


def _get_program(n_img, H, W):
    key = (n_img, H, W)
    if key not in _CACHE:
        _CACHE[key] = build_program(n_img, H, W)
    return _CACHE[key]


def _pack_inputs(pred, target):
    """pred/target [n_img, H, W] -> packed [npairs*H, 4*(W+2)]."""
    n_img, H, W = pred.shape
    assert n_img % 2 == 0
    npairs = n_img // 2
    S = W + 2
    out = np.zeros((npairs * H, 4 * S), dtype=np.float32)
    out[:, 1:W + 1] = pred[0::2].reshape(npairs * H, W)
    out[:, S + 1:S + W + 1] = pred[1::2].reshape(npairs * H, W)
    out[:, 2 * S + 1:2 * S + W + 1] = target[0::2].reshape(npairs * H, W)
    out[:, 3 * S + 1:3 * S + W + 1] = target[1::2].reshape(npairs * H, W)
    return out


LAST_RESULTS = None


def kernel(pred, target):
    from concourse.bass_utils import run_bass_kernel_spmd

    global LAST_RESULTS

    pred = np.asarray(pred, dtype=np.float32).reshape(16, IMG_H, IMG_W)
    target = np.asarray(target, dtype=np.float32).reshape(16, IMG_H, IMG_W)

    nc = _get_program(N_IMG_PER_CORE, IMG_H, IMG_W)
    bm = make_bmats(IMG_H)

    in_maps = []
    for c in range(N_CORES):
        sl = slice(c * N_IMG_PER_CORE, (c + 1) * N_IMG_PER_CORE)
        in_maps.append({
            "ptin": _pack_inputs(pred[sl], target[sl]),
            "bmats": bm,
        })

    res = run_bass_kernel_spmd(nc, in_maps, list(range(N_CORES)))
    LAST_RESULTS = res
    ssim_sum = 0.0
    l1_sum = 0.0
    for r in res.results:
        acc = r["acc_out"]
        ssim_sum += float(acc[:, 0].sum(dtype=np.float64))
        l1_sum += float(acc[:, 1].sum(dtype=np.float64))
    n = 16.0 * IMG_H * IMG_W
    loss = ALPHA * (ssim_sum / n) + BETA * (l1_sum / n)
    return np.float32(loss)



# revision 7
# speedup vs baseline: 3.6306x; 3.6306x over previous
"""SSIM(3x3 avg-pool) + L1 loss kernel for Trainium2, 8 NeuronCores.

loss = 0.85 * mean(clip((1 - ssim_map)/2, 0, 1)) + 0.15 * mean(|pred - target|)

Full inputs pred/target: (16, 1, 1024, 1024) f32. Data-parallel: 2 images per
core. On this execution path every instruction costs a ~flat 50-90us
regardless of size or engine (DRAM->SBUF DMA included), so the kernel
minimizes instruction count: no matmuls, no PSUM, no activation engine -
pure DVE + DMA, ~70 instructions per core.

Structure (per image pair):
  9 stripes of 128 output rows (stripe 8 all-zero padding; zero rows
  contribute 0 to both loss terms, so no masking). 3 groups of 3 stripes.
  Host packs ptin[130, 3, 12, 1026]: dim0 j = per-stripe image row offset
  j-1 (with halo, zeros outside the image), dim2 = group sections ordered
  [6 p-sections | 6 t-sections] (stripe-major, image-minor), each section
  zero-padded to 1026 cols.

  Per group (four [128, 12312] f32 buffers, 49.2KB/partition each):
    load copies A=rows-1, B=rows+0, C=rows+1 (3 partition-offset DMA views
    of the same array). L1 = |p-t| on copy B (rows exactly disjoint across
    stripes), accumulated via accum_out. Family 2 fields q=2pt, w=p^2+t^2
    computed per copy (custom SQSUM + one stt); vertical 3-tap = plain
    elementwise adds of the three copies (P1 = A+B+C for [p|t], P2 =
    qw(A)+qw(B)+qw(C)); horizontal 3-tap = 2 free-axis shifted adds per
    family. The /9 pool scale folds into the ssim constants (C -> 81C,
    cross terms x9 on B(2pt), B(p^2+t^2)).
    Post: T1 = [A2=2XY | V=X^2+Y^2] compact; ONE fused rational op computes
    [n1n2 | d1d2] in place (second operand = pooled [B(q)|B(w)] compact
    view); reciprocal; fused clip-accumulate into a per-group accumulator
    column. Host sums the accumulator slices.
"""

import sys

import numpy as np

sys.path.insert(0, "/opt/trn_rl_repo")

ALPHA = 0.85
BETA = 0.15
C1 = 0.01 ** 2
C2 = 0.03 ** 2

N_CORES = 8
IMG_H = 1024
IMG_W = 1024
N_IMG_PER_CORE = 2

BLK = 128                      # output rows per stripe (exact, halo via loads)
NS = 9                         # stripes per pair (stripe 8 = zeros)
KG = 3                         # stripes per group
NG = NS // KG                  # 3 groups
S = IMG_W + 2                  # padded section width (1026)
NSEC = 2 * KG                  # sections per half (6)
HW_ = NSEC * S                 # half width (6156)
WID = 2 * HW_                  # flat tile width (12312)
CW = NSEC * IMG_W              # compact half width (6144)

# scaled ssim constants (pooled fields carry a 9x box-sum scale)
SC1 = 81.0 * C1
SC2 = 81.0 * C2
SXS = 9.0                      # n2*81 = 9*B(2pt) - A2' + 81C2 (d2 likewise)

# --- custom fused DVE ops ------------------------------------------------- #
_OP_SQSUM = None       # out = in0^2 + in1^2
_OP_SSIM_RAT = None    # out = (in0 + s0) * (in1*s1 - in0 + imm2)
_OP_SSIM_FINAL = None  # out = (s0 - clamp(in0*in1, s1, s0))*imm2; accum += out
_OP_ABSD = None        # out = |in0 - in1|; accum += out
_CUSTOM_OPS_OK = False


def _register_custom_ops():
    global _OP_SQSUM, _OP_SSIM_RAT, _OP_SSIM_FINAL, _OP_ABSD, _CUSTOM_OPS_OK
    if _CUSTOM_OPS_OK:
        return
    from operator import add

    import concourse.dve_ops as dv
    from concourse.dve_spec import (
        C0, C1 as KC1, C2 as KC2, AluOp, Bin, Spec, Src0, Src1, Zero,
        lower, maxx, minn, sq,
    )
    from concourse.dve_uop import DveOpSpec

    def _sqsum_ref(in0, in1, c0, c1, c2):
        return in0.astype(np.float32) ** 2 + in1.astype(np.float32) ** 2

    def _rat_ref(in0, in1, c0, c1, c2):
        a = in0.astype(np.float32)
        return (a + c0) * (in1.astype(np.float32) * c1 - a + c2)

    def _final_ref(in0, in1, c0, c1, c2):
        z = in0.astype(np.float32) * in1.astype(np.float32)
        b = ((c0 - np.clip(z, c1, c0)) * c2).astype(np.float32)
        return b, b.reshape(b.shape[0], -1).sum(axis=-1, keepdims=True)

    def _absd_ref(in0, in1, c0, c1, c2):
        b = np.abs(in0.astype(np.float32) - in1.astype(np.float32))
        return b, b.reshape(b.shape[0], -1).sum(axis=-1, keepdims=True)

    defs = [
        ("SSIM_SQSUM_ANT", Spec(body=sq(Src0) + sq(Src1), reference=_sqsum_ref)),
        ("SSIM_RAT_ANT", Spec(
            body=(Src0 + C0) * (Src1 * KC1 - Src0 + KC2), reference=_rat_ref)),
        ("SSIM_FINAL_ANT", Spec(
            body=(C0 - maxx(minn(Src0 * Src1, C0), KC1)) * KC2,
            accum=add, accum_init=Zero, reference=_final_ref)),
        ("SSIM_ABSD_ANT", Spec(
            body=Bin(AluOp.ABSOLUTE_DIFF, Src0, Src1),
            accum=add, accum_init=Zero, reference=_absd_ref)),
    ]
    made = {}
    for name, spec in defs:
        if name not in dv._SUB_OPCODE_FOR_NAME:
            stub = dv.DveOp(name, spec, subdim=False, uops_sha={})
            dv.OPS.append(stub)
            dv._SUB_OPCODE_FOR_NAME[name] = (
                dv._CUSTOM_DVE_ROW_BASE + len(dv.OPS) - 1
            )
            dv.CUSTOM_DVE_SPECS[name] = spec
        opcode = dv._SUB_OPCODE_FOR_NAME[name]
        shas = {}
        for ver in ("v3", "v4"):
            res = DveOpSpec(
                name=name, opcode=opcode, uops=lower(spec, ver=ver),
                rd1_en=dv.has_src1(spec),
            )
            shas[ver] = res.sha(ver)
        op = dv.DveOp(name, spec, subdim=False, uops_sha=shas)
        idx = next(i for i, o in enumerate(dv.OPS) if o.name == name)
        dv.OPS[idx] = op
        dv.CUSTOM_DVE_SPECS[name] = spec
        made[name] = op
    _OP_SQSUM = made["SSIM_SQSUM_ANT"]
    _OP_SSIM_RAT = made["SSIM_RAT_ANT"]
    _OP_SSIM_FINAL = made["SSIM_FINAL_ANT"]
    _OP_ABSD = made["SSIM_ABSD_ANT"]
    _CUSTOM_OPS_OK = True


def build_program(n_img, H, W, io_internal=False):
    """Per-core program for n_img (even) HxW images.

    DRAM input "ptin": [130, npairs*NG, 2*NSEC, S] f32 (see module doc).
    Output "acc_out": [128, 8*npairs]; per pair p columns 8p+{0,1,2}: L1
    partials (partitions 0:128, one per group), 8p+{4,5,6}: ssim partials.
    """
    import concourse.bacc as bacc
    import concourse.tile as tile
    from concourse import mybir

    assert n_img % 2 == 0
    f32 = mybir.dt.float32
    Alu = mybir.AluOpType
    npairs = n_img // 2

    _register_custom_ops()
    nc = bacc.Bacc("TRN2", target_bir_lowering=False, debug=False)

    io_kind = "Internal" if io_internal else "ExternalInput"
    ptin_d = nc.dram_tensor(
        "ptin", [130, npairs * NG, 2 * NSEC, S], f32, kind=io_kind).ap()
    acc_d = nc.dram_tensor(
        "acc_out", [128, 8 * npairs], f32, kind="ExternalOutput").ap()

    def secv(t):
        # [128, 2*NSEC, S] section view of a flat [128, WID] tile
        return t[:, :].rearrange("p (f c) -> p f c", f=2 * NSEC, c=S)

    def compv(t, p0, p1):
        # [p0:p1, 2*NSEC, W] compact view (stride W) of the leading cols
        return t[:, 0:2 * NSEC * W].rearrange(
            "p (f c) -> p f c", f=2 * NSEC, c=W)[p0:p1, :, :]

    with tile.TileContext(nc) as tc:
        with (
            tc.tile_pool(name="buf1", bufs=1) as pool1,
            tc.tile_pool(name="buf2", bufs=1) as pool2,
            tc.tile_pool(name="buf3", bufs=1) as pool3,
            tc.tile_pool(name="buf4", bufs=1) as pool4,
            tc.tile_pool(name="misc", bufs=1) as mpool,
        ):
            acc = mpool.tile([128, 8 * npairs], f32, tag="acc")

            for pair in range(npairs):
                gbase = pair * NG
                cbase = pair * 8
                for g in range(NG):
                    gi = gbase + g

                    def load(pool, nm, off):
                        t = pool.tile([128, WID], f32, tag=nm[0], name=nm)
                        nc.sync.dma_start(
                            out=secv(t),
                            in_=ptin_d[off:off + 128, gi, :, :])
                        return t

                    def qw(dst, src):
                        # dst = [q=2pt | w=p^2+t^2] from src's [p|t] halves
                        nc.vector.scalar_tensor_tensor(
                            dst[:, 0:HW_], src[:, 0:HW_], 2.0,
                            src[:, HW_:WID], op0=Alu.mult, op1=Alu.mult)
                        nc.vector._custom_dve(
                            _OP_SQSUM, out=dst[:, HW_:WID],
                            in0=src[:, 0:HW_], in1=src[:, HW_:WID])

                    A = load(pool1, "1A", 0)
                    qwA = pool2.tile([128, WID], f32, tag="2", name="qwA")
                    qw(qwA, A)
                    B = load(pool3, "3B", 1)
                    # L1 on copy B: rows 128s..128s+127, disjoint across all
                    # stripes; dump values into buf4 (dead region)
                    dmp = pool4.tile([128, WID], f32, tag="4", name="dmp")
                    nc.vector._custom_dve(
                        _OP_ABSD, out=dmp[:, HW_:WID],
                        in0=B[:, 0:HW_], in1=B[:, HW_:WID],
                        accum_out=acc[:, cbase + g: cbase + g + 1])
                    qwB = dmp  # reuse the same buf4 instance
                    qw(qwB, B)
                    nc.vector.tensor_add(qwA[:, :], qwA[:, :], qwB[:, :])
                    nc.vector.tensor_add(A[:, :], A[:, :], B[:, :])
                    C = load(pool4, "4C", 2)
                    qwC = pool3.tile([128, WID], f32, tag="3", name="qwC")
                    qw(qwC, C)
                    nc.vector.tensor_add(qwA[:, :], qwA[:, :], qwC[:, :])
                    nc.vector.tensor_add(A[:, :], A[:, :], C[:, :])
                    # now A = P1 = Bv([p|t]) (buf1), qwA = P2 = Bv([q|w]) (buf2)

                    # horizontal 3-tap, written COMPACT (3D section-sliced
                    # adds drop each section's 2 pad cols at the source)
                    As = secv(A)
                    H1 = pool4.tile([128, WID], f32, tag="4", name="H1")
                    H1c = compv(H1, 0, 128)
                    nc.vector.tensor_add(
                        H1c, As[:, :, 0:W], As[:, :, 1:W + 1])
                    nc.vector.tensor_add(H1c, H1c, As[:, :, 2:W + 2])
                    qs = secv(qwA)
                    H2 = pool3.tile([128, WID], f32, tag="3", name="H2")
                    H2c = compv(H2, 0, 128)
                    nc.vector.tensor_add(
                        H2c, qs[:, :, 0:W], qs[:, :, 1:W + 1])
                    nc.vector.tensor_add(H2c, H2c, qs[:, :, 2:W + 2])

                    # post-pool on flat compact views: T1 = [A2=2XY | V]
                    X = H1[:, 0:CW]
                    Y = H1[:, CW:2 * CW]
                    T1 = pool1.tile([128, WID], f32, tag="1", name="T1")
                    nc.vector.scalar_tensor_tensor(
                        T1[:, 0:CW], X, 2.0, Y,
                        op0=Alu.mult, op1=Alu.mult)
                    nc.vector._custom_dve(
                        _OP_SQSUM, out=T1[:, CW:2 * CW], in0=X, in1=Y)
                    # fused rationals vs pooled [B(q)|B(w)] compact halves:
                    # R = (T1+SC1)*(H2*SXS - T1 + SC2) in place -> [n1n2|d1d2]
                    nc.vector._custom_dve(
                        _OP_SSIM_RAT, out=T1[:, 0:2 * CW], in0=T1[:, 0:2 * CW],
                        in1=H2[:, 0:2 * CW], s0=SC1, s1=SXS, imm2=SC2)
                    # reciprocal of d1d2 -> buf2 (P2 dead)
                    Rc = pool2.tile([128, WID], f32, tag="2", name="Rc")
                    nc.vector.reciprocal_approx_fast(
                        Rc[0:128, 0:CW], T1[0:128, CW:2 * CW])
                    # fin = (1 - clamp(n1n2 * rcp, -1, 1)) * 0.5, accum
                    nc.vector._custom_dve(
                        _OP_SSIM_FINAL, out=Rc[0:128, CW:2 * CW],
                        in0=T1[0:128, 0:CW], in1=Rc[0:128, 0:CW],
                        s0=1.0, s1=-1.0, imm2=0.5,
                        accum_out=acc[0:128, cbase + 4 + g: cbase + 5 + g])

            nc.sync.dma_start(out=acc_d[:, :], in_=acc[:, :])

    nc.compile()
    return nc


_CACHE = {}


def _get_program(n_img, H, W):
    key = (n_img, H, W)
    if key not in _CACHE:
        _CACHE[key] = build_program(n_img, H, W)
    return _CACHE[key]


def make_bmats(H):
    """Compat stub for older harnesses (no matmuls in this kernel)."""
    return np.zeros((1, 1), dtype=np.float32)


def _pack_inputs(pred, target):
    """pred/target [n_img, H, W] -> packed [130, npairs*NG, 2*NSEC, S]."""
    n_img, H, W = pred.shape
    assert n_img % 2 == 0
    npairs = n_img // 2
    out = np.zeros((130, npairs * NG, 2 * NSEC, S), dtype=np.float32)
    # padded row store: index r+1 = image row r, zeros outside
    pad_h = BLK * (NS - 1) + 130
    # dram j, stripe s -> padded row index 128*s + j (j=0 -> image row -1)
    J = (BLK * np.arange(NS)[None, :] + np.arange(130)[:, None])  # [130, NS]
    for pair in range(npairs):
        fields = (pred[2 * pair], pred[2 * pair + 1],
                  target[2 * pair], target[2 * pair + 1])
        for half in range(2):  # 0: p, 1: t
            for img in range(2):
                Pimg = np.zeros((pad_h, W), dtype=np.float32)
                Pimg[1:H + 1] = fields[2 * half + img]
                R = Pimg[J]  # [130, NS, W]
                for g in range(NG):
                    for s in range(KG):
                        out[:, pair * NG + g, half * NSEC + 2 * s + img,
                            1:W + 1] = R[:, g * KG + s]
    return out


LAST_RESULTS = None


def kernel(pred, target):
    from concourse.bass_utils import run_bass_kernel_spmd

    global LAST_RESULTS

    pred = np.asarray(pred, dtype=np.float32).reshape(16, IMG_H, IMG_W)
    target = np.asarray(target, dtype=np.float32).reshape(16, IMG_H, IMG_W)

    nc = _get_program(N_IMG_PER_CORE, IMG_H, IMG_W)

    in_maps = []
    for c in range(N_CORES):
        sl = slice(c * N_IMG_PER_CORE, (c + 1) * N_IMG_PER_CORE)
        in_maps.append({"ptin": _pack_inputs(pred[sl], target[sl])})

    res = run_bass_kernel_spmd(nc, in_maps, list(range(N_CORES)))
    LAST_RESULTS = res
    npairs = N_IMG_PER_CORE // 2
    ssim_sum = 0.0
    l1_sum = 0.0
    for r in res.results:
        acc = r["acc_out"]
        for p in range(npairs):
            b = 8 * p
            l1_sum += float(acc[:, b:b + 3].sum(dtype=np.float64))
            ssim_sum += float(acc[:, b + 4:b + 7].sum(dtype=np.float64))
    n = 16.0 * IMG_H * IMG_W
    loss = ALPHA * (ssim_sum / n) + BETA * (l1_sum / n)
    return np.float32(loss)


# revision 8
# speedup vs baseline: 3.9603x; 1.0908x over previous
"""SSIM(3x3 avg-pool) + L1 loss kernel for Trainium2, 8 NeuronCores.

loss = 0.85 * mean(clip((1 - ssim_map)/2, 0, 1)) + 0.15 * mean(|pred - target|)

Full inputs pred/target: (16, 1, 1024, 1024) f32. Data-parallel: 2 images per
core. On this execution path every instruction costs a ~flat 50-90us
regardless of size or engine (DRAM->SBUF DMA included), so the kernel
minimizes instruction count: no matmuls, no PSUM, no activation engine -
pure DVE + DMA, ~70 instructions per core.

Structure (per image pair):
  9 stripes of 128 output rows (stripe 8 all-zero padding; zero rows
  contribute 0 to both loss terms, so no masking). 3 groups of 3 stripes.
  Host packs ptin[130, 3, 12, 1026]: dim0 j = per-stripe image row offset
  j-1 (with halo, zeros outside the image), dim2 = group sections ordered
  [6 p-sections | 6 t-sections] (stripe-major, image-minor), each section
  zero-padded to 1026 cols.

  Per group (four [128, 12312] f32 buffers, 49.2KB/partition each):
    load copies A=rows-1, B=rows+0, C=rows+1 (3 partition-offset DMA views
    of the same array). L1 = |p-t| on copy B (rows exactly disjoint across
    stripes), accumulated via accum_out. Family 2 fields q=2pt, w=p^2+t^2
    computed per copy (custom SQSUM + one stt); vertical 3-tap = plain
    elementwise adds of the three copies (P1 = A+B+C for [p|t], P2 =
    qw(A)+qw(B)+qw(C)); horizontal 3-tap = 2 free-axis shifted adds per
    family. The /9 pool scale folds into the ssim constants (C -> 81C,
    cross terms x9 on B(2pt), B(p^2+t^2)).
    Post: T1 = [A2=2XY | V=X^2+Y^2] compact; ONE fused rational op computes
    [n1n2 | d1d2] in place (second operand = pooled [B(q)|B(w)] compact
    view); reciprocal; fused clip-accumulate into a per-group accumulator
    column. Host sums the accumulator slices.
"""

import sys

import numpy as np

sys.path.insert(0, "/opt/trn_rl_repo")

ALPHA = 0.85
BETA = 0.15
C1 = 0.01 ** 2
C2 = 0.03 ** 2

N_CORES = 8
IMG_H = 1024
IMG_W = 1024
N_IMG_PER_CORE = 2

BLK = 128                      # output rows per stripe (exact, halo via loads)
NS = 9                         # stripes per pair (stripe 8 = zeros)
KG = 3                         # stripes per group
NG = NS // KG                  # 3 groups
S = IMG_W + 2                  # padded section width (1026)
NSEC = 2 * KG                  # sections per half (6)
HW_ = NSEC * S                 # half width (6156)
WID = 2 * HW_                  # flat tile width (12312)
CW = NSEC * IMG_W              # compact half width (6144)

# scaled ssim constants (pooled fields carry a 9x box-sum scale)
SC1 = 81.0 * C1
SC2 = 81.0 * C2
SXS = 9.0                      # n2*81 = 9*B(2pt) - A2' + 81C2 (d2 likewise)

# --- custom fused DVE ops ------------------------------------------------- #
_OP_SQSUM = None       # out = in0^2 + in1^2
_OP_SSIM_RAT = None    # out = (in0 + s0) * (in1*s1 - in0 + imm2)
_OP_SSIM_FINAL = None  # out = (s0 - clamp(in0*in1, s1, s0))*imm2; accum += out
_OP_ABSD = None        # out = |in0 - in1|; accum += out
_CUSTOM_OPS_OK = False


def _register_custom_ops():
    global _OP_SQSUM, _OP_SSIM_RAT, _OP_SSIM_FINAL, _OP_ABSD, _CUSTOM_OPS_OK
    if _CUSTOM_OPS_OK:
        return
    from operator import add

    import concourse.dve_ops as dv
    from concourse.dve_spec import (
        C0, C1 as KC1, C2 as KC2, AluOp, Bin, Spec, Src0, Src1, Zero,
        lower, maxx, minn, sq,
    )
    from concourse.dve_uop import DveOpSpec

    def _sqsum_ref(in0, in1, c0, c1, c2):
        return in0.astype(np.float32) ** 2 + in1.astype(np.float32) ** 2

    def _rat_ref(in0, in1, c0, c1, c2):
        a = in0.astype(np.float32)
        return (a + c0) * (in1.astype(np.float32) * c1 - a + c2)

    def _final_ref(in0, in1, c0, c1, c2):
        z = in0.astype(np.float32) * in1.astype(np.float32)
        b = ((c0 - np.clip(z, c1, c0)) * c2).astype(np.float32)
        return b, b.reshape(b.shape[0], -1).sum(axis=-1, keepdims=True)

    def _absd_ref(in0, in1, c0, c1, c2):
        b = np.abs(in0.astype(np.float32) - in1.astype(np.float32))
        return b, b.reshape(b.shape[0], -1).sum(axis=-1, keepdims=True)

    defs = [
        ("SSIM_SQSUM_ANT", Spec(body=sq(Src0) + sq(Src1), reference=_sqsum_ref)),
        ("SSIM_RAT_ANT", Spec(
            body=(Src0 + C0) * (Src1 * KC1 - Src0 + KC2), reference=_rat_ref)),
        ("SSIM_FINAL_ANT", Spec(
            body=(C0 - maxx(minn(Src0 * Src1, C0), KC1)) * KC2,
            accum=add, accum_init=Zero, reference=_final_ref)),
        ("SSIM_ABSD_ANT", Spec(
            body=Bin(AluOp.ABSOLUTE_DIFF, Src0, Src1),
            accum=add, accum_init=Zero, reference=_absd_ref)),
    ]
    made = {}
    for name, spec in defs:
        if name not in dv._SUB_OPCODE_FOR_NAME:
            stub = dv.DveOp(name, spec, subdim=False, uops_sha={})
            dv.OPS.append(stub)
            dv._SUB_OPCODE_FOR_NAME[name] = (
                dv._CUSTOM_DVE_ROW_BASE + len(dv.OPS) - 1
            )
            dv.CUSTOM_DVE_SPECS[name] = spec
        opcode = dv._SUB_OPCODE_FOR_NAME[name]
        shas = {}
        for ver in ("v3", "v4"):
            res = DveOpSpec(
                name=name, opcode=opcode, uops=lower(spec, ver=ver),
                rd1_en=dv.has_src1(spec),
            )
            shas[ver] = res.sha(ver)
        op = dv.DveOp(name, spec, subdim=False, uops_sha=shas)
        idx = next(i for i, o in enumerate(dv.OPS) if o.name == name)
        dv.OPS[idx] = op
        dv.CUSTOM_DVE_SPECS[name] = spec
        made[name] = op
    _OP_SQSUM = made["SSIM_SQSUM_ANT"]
    _OP_SSIM_RAT = made["SSIM_RAT_ANT"]
    _OP_SSIM_FINAL = made["SSIM_FINAL_ANT"]
    _OP_ABSD = made["SSIM_ABSD_ANT"]
    _CUSTOM_OPS_OK = True


def build_program(n_img, H, W, io_internal=False):
    """Per-core program for n_img (even) HxW images.

    DRAM input "ptin": [130, npairs*NG, 2*NSEC, S] f32 (see module doc).
    Output "acc_out": [128, 8*npairs]; per pair p columns 8p+{0,1,2}: L1
    partials (partitions 0:128, one per group), 8p+{4,5,6}: ssim partials.
    """
    import concourse.bacc as bacc
    import concourse.tile as tile
    from concourse import mybir

    assert n_img % 2 == 0
    f32 = mybir.dt.float32
    Alu = mybir.AluOpType
    npairs = n_img // 2

    _register_custom_ops()
    nc = bacc.Bacc("TRN2", target_bir_lowering=False, debug=False)

    io_kind = "Internal" if io_internal else "ExternalInput"
    ptin_d = nc.dram_tensor(
        "ptin", [130, npairs * NG, 2 * NSEC, S], f32, kind=io_kind).ap()
    acc_d = nc.dram_tensor(
        "acc_out", [128, 8 * npairs], f32, kind="ExternalOutput").ap()

    def secv(t):
        # [128, 2*NSEC, S] section view of a flat [128, WID] tile
        return t[:, :].rearrange("p (f c) -> p f c", f=2 * NSEC, c=S)

    def compv(t, p0, p1):
        # [p0:p1, 2*NSEC, W] compact view (stride W) of the leading cols
        return t[:, 0:2 * NSEC * W].rearrange(
            "p (f c) -> p f c", f=2 * NSEC, c=W)[p0:p1, :, :]

    DW = 2 * WID  # double-wide: [p|t|q|w] families fused (24624 f32)

    with tile.TileContext(nc) as tc:
        with (
            tc.tile_pool(name="bufP", bufs=1) as poolP,
            tc.tile_pool(name="bufS", bufs=1) as poolS,
            tc.tile_pool(name="misc", bufs=1) as mpool,
        ):
            acc = mpool.tile([128, 8 * npairs], f32, tag="acc")

            for pair in range(npairs):
                gbase = pair * NG
                cbase = pair * 8
                for g in range(NG):
                    gi = gbase + g

                    def qw(t):
                        # t[q|w] = [2pt | p^2+t^2] from t's [p|t] halves
                        nc.vector.scalar_tensor_tensor(
                            t[:, WID:WID + HW_], t[:, 0:HW_], 2.0,
                            t[:, HW_:WID], op0=Alu.mult, op1=Alu.mult)
                        nc.vector._custom_dve(
                            _OP_SQSUM, out=t[:, WID + HW_:DW],
                            in0=t[:, 0:HW_], in1=t[:, HW_:WID])

                    # accumulator tile: copy A loads into the [p|t] half,
                    # its qw fields are computed in place alongside
                    P = poolP.tile([128, DW], f32, tag="P", name="P")
                    nc.sync.dma_start(
                        out=P[:, 0:WID].rearrange(
                            "p (f c) -> p f c", f=2 * NSEC, c=S),
                        in_=ptin_d[0:128, gi, :, :])
                    qw(P)
                    # stage tile: copy B + its qw, then ONE double-wide add
                    SB = poolS.tile([128, DW], f32, tag="S", name="SB")
                    nc.sync.dma_start(
                        out=SB[:, 0:WID].rearrange(
                            "p (f c) -> p f c", f=2 * NSEC, c=S),
                        in_=ptin_d[1:129, gi, :, :])
                    # L1 on copy B (rows 128s..128s+127, disjoint across
                    # stripes); dump into SB's q|w half (overwritten next)
                    nc.vector._custom_dve(
                        _OP_ABSD, out=SB[:, WID:WID + HW_],
                        in0=SB[:, 0:HW_], in1=SB[:, HW_:WID],
                        accum_out=acc[:, cbase + g: cbase + g + 1])
                    qw(SB)
                    nc.vector.tensor_add(P[:, :], P[:, :], SB[:, :])
                    # copy C likewise
                    SC_ = poolS.tile([128, DW], f32, tag="S", name="SC")
                    nc.sync.dma_start(
                        out=SC_[:, 0:WID].rearrange(
                            "p (f c) -> p f c", f=2 * NSEC, c=S),
                        in_=ptin_d[2:130, gi, :, :])
                    qw(SC_)
                    nc.vector.tensor_add(P[:, :], P[:, :], SC_[:, :])
                    # P = Bv([p|t|q|w]) (vertical 3-tap done)

                    # horizontal 3-tap for all 4 families at once, written
                    # COMPACT (3D section-sliced adds drop the pad cols)
                    Ps = P[:, :].rearrange("p (f c) -> p f c", f=4 * NSEC, c=S)
                    Ht = poolS.tile([128, DW], f32, tag="S", name="Ht")
                    Hc = Ht[:, 0:4 * NSEC * W].rearrange(
                        "p (f c) -> p f c", f=4 * NSEC, c=W)
                    nc.vector.tensor_add(
                        Hc, Ps[:, :, 0:W], Ps[:, :, 1:W + 1])
                    nc.vector.tensor_add(Hc, Hc, Ps[:, :, 2:W + 2])
                    # Ht compact: [X | Y | B(q) | B(w)] at CW each

                    # post-pool on flat views: T = [A2=2XY | V=X^2+Y^2]
                    X = Ht[:, 0:CW]
                    Y = Ht[:, CW:2 * CW]
                    T = poolP.tile([128, DW], f32, tag="P", name="T")
                    nc.vector.scalar_tensor_tensor(
                        T[:, 0:CW], X, 2.0, Y, op0=Alu.mult, op1=Alu.mult)
                    nc.vector._custom_dve(
                        _OP_SQSUM, out=T[:, CW:2 * CW], in0=X, in1=Y)
                    # fused rationals: R = (T+SC1)*(Hqw*SXS - T + SC2)
                    # in place -> [n1n2 | d1d2]
                    nc.vector._custom_dve(
                        _OP_SSIM_RAT, out=T[:, 0:2 * CW], in0=T[:, 0:2 * CW],
                        in1=Ht[:, 2 * CW:4 * CW], s0=SC1, s1=SXS, imm2=SC2)
                    # reciprocal of d1d2, then fused clip-accumulate
                    nc.vector.reciprocal_approx_fast(
                        T[:, 2 * CW:3 * CW], T[:, CW:2 * CW])
                    nc.vector._custom_dve(
                        _OP_SSIM_FINAL, out=T[:, 3 * CW:4 * CW],
                        in0=T[:, 0:CW], in1=T[:, 2 * CW:3 * CW],
                        s0=1.0, s1=-1.0, imm2=0.5,
                        accum_out=acc[:, cbase + 4 + g: cbase + 5 + g])

            nc.sync.dma_start(out=acc_d[:, :], in_=acc[:, :])

    nc.compile()
    return nc


_CACHE = {}


def _get_program(n_img, H, W):
    key = (n_img, H, W)
    if key not in _CACHE:
        _CACHE[key] = build_program(n_img, H, W)
    return _CACHE[key]


def make_bmats(H):
    """Compat stub for older harnesses (no matmuls in this kernel)."""
    return np.zeros((1, 1), dtype=np.float32)


def _pack_inputs(pred, target):
    """pred/target [n_img, H, W] -> packed [130, npairs*NG, 2*NSEC, S]."""
    n_img, H, W = pred.shape
    assert n_img % 2 == 0
    npairs = n_img // 2
    out = np.zeros((130, npairs * NG, 2 * NSEC, S), dtype=np.float32)
    # padded row store: index r+1 = image row r, zeros outside
    pad_h = BLK * (NS - 1) + 130
    # dram j, stripe s -> padded row index 128*s + j (j=0 -> image row -1)
    J = (BLK * np.arange(NS)[None, :] + np.arange(130)[:, None])  # [130, NS]
    for pair in range(npairs):
        fields = (pred[2 * pair], pred[2 * pair + 1],
                  target[2 * pair], target[2 * pair + 1])
        for half in range(2):  # 0: p, 1: t
            for img in range(2):
                Pimg = np.zeros((pad_h, W), dtype=np.float32)
                Pimg[1:H + 1] = fields[2 * half + img]
                R = Pimg[J]  # [130, NS, W]
                for g in range(NG):
                    for s in range(KG):
                        out[:, pair * NG + g, half * NSEC + 2 * s + img,
                            1:W + 1] = R[:, g * KG + s]
    return out


LAST_RESULTS = None


def kernel(pred, target):
    from concourse.bass_utils import run_bass_kernel_spmd

    global LAST_RESULTS

    pred = np.asarray(pred, dtype=np.float32).reshape(16, IMG_H, IMG_W)
    target = np.asarray(target, dtype=np.float32).reshape(16, IMG_H, IMG_W)

    nc = _get_program(N_IMG_PER_CORE, IMG_H, IMG_W)

    in_maps = []
    for c in range(N_CORES):
        sl = slice(c * N_IMG_PER_CORE, (c + 1) * N_IMG_PER_CORE)
        in_maps.append({"ptin": _pack_inputs(pred[sl], target[sl])})

    res = run_bass_kernel_spmd(nc, in_maps, list(range(N_CORES)))
    LAST_RESULTS = res
    npairs = N_IMG_PER_CORE // 2
    ssim_sum = 0.0
    l1_sum = 0.0
    for r in res.results:
        acc = r["acc_out"]
        for p in range(npairs):
            b = 8 * p
            l1_sum += float(acc[:, b:b + 3].sum(dtype=np.float64))
            ssim_sum += float(acc[:, b + 4:b + 7].sum(dtype=np.float64))
    n = 16.0 * IMG_H * IMG_W
    loss = ALPHA * (ssim_sum / n) + BETA * (l1_sum / n)
    return np.float32(loss)


# revision 9
# speedup vs baseline: 5.2420x; 1.3236x over previous
"""SSIM(3x3 avg-pool) + L1 loss kernel for Trainium2, 8 NeuronCores.

loss = 0.85 * mean(clip((1 - ssim_map)/2, 0, 1)) + 0.15 * mean(|pred - target|)

Full inputs pred/target: (16, 1, 1024, 1024) f32. Data-parallel: 2 images per
core. On this execution path every instruction costs a ~flat 50-90us
regardless of size or engine (DRAM->SBUF DMA included), so the kernel
minimizes instruction count: no matmuls, no PSUM, no activation engine -
pure DVE + DMA, ~41 instructions per core.

Structure (per image pair):
  8 stripes of 128 output rows, 2 groups of 4 stripes. Host packs
  ptin[130, 2, 16, 1026] bf16: dim0 j = per-stripe image row offset j-1
  (halo rows, zeros outside the image), dim2 = group sections ordered
  [8 p-sections | 8 t-sections] (stripe-major, image-minor), zero-padded
  to 1026 cols. bf16 staging only touches the raw inputs and per-copy
  q=2pt / w=p^2+t^2 products (~0.2% input rounding, bias ~1e-4); all
  pooling accumulation and post-pool math is f32.

  Per group (P: [128, 32832] f32 accumulator tile, S: [128, 32832] bf16
  stage tile - 197KB/partition total):
    load copy A (row offset -1) -> S.[p|t], q/w -> S.[q|w] (stt + custom
    sqsum), P = copy(S) widening to f32; copy B (offset 0): load, L1 =
    |p-t| via custom absdiff accum (rows exactly disjoint across stripes),
    q/w, P += S (one double-wide mixed-dtype add); copy C (offset +1)
    likewise. P now holds the vertical 3-tap of [p|t|q|w].
    Horizontal 3-tap written compact f32: H.pt -> S-buffer (f32 instance),
    T=[A2=2XY | V=X^2+Y^2] -> P's dead pt region, H.qw -> S-buffer, then
    ONE fused rational op R=(T+81C1)*(Hqw*9 - T + 81C2) -> [n1n2|d1d2]
    in place, reciprocal, fused clip-accumulate into per-group accumulator
    columns (the /9 pool scale is folded into the constants).
  Host sums the accumulator slices.
"""

import sys

import numpy as np

sys.path.insert(0, "/opt/trn_rl_repo")

ALPHA = 0.85
BETA = 0.15
C1 = 0.01 ** 2
C2 = 0.03 ** 2

N_CORES = 8
IMG_H = 1024
IMG_W = 1024
N_IMG_PER_CORE = 2

BLK = 128                      # output rows per stripe (halo via loads)
NS = 8                         # stripes per pair (exact: 8*128 = 1024)
KG = 4                         # stripes per group
NG = NS // KG                  # 2 groups
S = IMG_W + 2                  # padded section width (1026)
NSEC = 2 * KG                  # sections per half (8)
HW_ = NSEC * S                 # half width (8208)
WID = 2 * HW_                  # [p|t] width (16416)
DW = 2 * WID                   # [p|t|q|w] width (32832)
CW = NSEC * IMG_W              # compact half width (8192)

# scaled ssim constants (pooled fields carry a 9x box-sum scale)
SC1 = 81.0 * C1
SC2 = 81.0 * C2
SXS = 9.0                      # n2*81 = 9*B(2pt) - A2' + 81C2 (d2 likewise)

# --- custom fused DVE ops ------------------------------------------------- #
_OP_SQSUM = None       # out = in0^2 + in1^2
_OP_SSIM_RAT = None    # out = (in0 + s0) * (in1*s1 - in0 + imm2)
_OP_SSIM_FINAL = None  # out = (s0 - clamp(in0*in1, s1, s0))*imm2; accum += out
_OP_ABSD = None        # out = |in0 - in1|; accum += out
_CUSTOM_OPS_OK = False


def _register_custom_ops():
    global _OP_SQSUM, _OP_SSIM_RAT, _OP_SSIM_FINAL, _OP_ABSD, _CUSTOM_OPS_OK
    if _CUSTOM_OPS_OK:
        return
    from operator import add

    import concourse.dve_ops as dv
    from concourse.dve_spec import (
        C0, C1 as KC1, C2 as KC2, AluOp, Bin, Spec, Src0, Src1, Zero,
        lower, maxx, minn, sq,
    )
    from concourse.dve_uop import DveOpSpec

    def _sqsum_ref(in0, in1, c0, c1, c2):
        return in0.astype(np.float32) ** 2 + in1.astype(np.float32) ** 2

    def _rat_ref(in0, in1, c0, c1, c2):
        a = in0.astype(np.float32)
        return (a + c0) * (in1.astype(np.float32) * c1 - a + c2)

    def _final_ref(in0, in1, c0, c1, c2):
        z = in0.astype(np.float32) * in1.astype(np.float32)
        b = ((c0 - np.clip(z, c1, c0)) * c2).astype(np.float32)
        return b, b.reshape(b.shape[0], -1).sum(axis=-1, keepdims=True)

    def _absd_ref(in0, in1, c0, c1, c2):
        b = np.abs(in0.astype(np.float32) - in1.astype(np.float32))
        return b, b.reshape(b.shape[0], -1).sum(axis=-1, keepdims=True)

    defs = [
        ("SSIM_SQSUM_ANT", Spec(body=sq(Src0) + sq(Src1), reference=_sqsum_ref)),
        ("SSIM_RAT_ANT", Spec(
            body=(Src0 + C0) * (Src1 * KC1 - Src0 + KC2), reference=_rat_ref)),
        ("SSIM_FINAL_ANT", Spec(
            body=(C0 - maxx(minn(Src0 * Src1, C0), KC1)) * KC2,
            accum=add, accum_init=Zero, reference=_final_ref)),
        ("SSIM_ABSD_ANT", Spec(
            body=Bin(AluOp.ABSOLUTE_DIFF, Src0, Src1),
            accum=add, accum_init=Zero, reference=_absd_ref)),
    ]
    made = {}
    for name, spec in defs:
        if name not in dv._SUB_OPCODE_FOR_NAME:
            stub = dv.DveOp(name, spec, subdim=False, uops_sha={})
            dv.OPS.append(stub)
            dv._SUB_OPCODE_FOR_NAME[name] = (
                dv._CUSTOM_DVE_ROW_BASE + len(dv.OPS) - 1
            )
            dv.CUSTOM_DVE_SPECS[name] = spec
        opcode = dv._SUB_OPCODE_FOR_NAME[name]
        shas = {}
        for ver in ("v3", "v4"):
            res = DveOpSpec(
                name=name, opcode=opcode, uops=lower(spec, ver=ver),
                rd1_en=dv.has_src1(spec),
            )
            shas[ver] = res.sha(ver)
        op = dv.DveOp(name, spec, subdim=False, uops_sha=shas)
        idx = next(i for i, o in enumerate(dv.OPS) if o.name == name)
        dv.OPS[idx] = op
        dv.CUSTOM_DVE_SPECS[name] = spec
        made[name] = op
    _OP_SQSUM = made["SSIM_SQSUM_ANT"]
    _OP_SSIM_RAT = made["SSIM_RAT_ANT"]
    _OP_SSIM_FINAL = made["SSIM_FINAL_ANT"]
    _OP_ABSD = made["SSIM_ABSD_ANT"]
    _CUSTOM_OPS_OK = True


def build_program(n_img, H, W, io_internal=False):
    """Per-core program for n_img (even) HxW images.

    DRAM input "ptin": [130, npairs*NG, 2*NSEC, S] bf16 (see module doc).
    Output "acc_out": [128, 8*npairs]; per pair p columns 8p+{0,1}: L1
    partials (one per group), 8p+{4,5}: ssim partials.
    """
    import concourse.bacc as bacc
    import concourse.tile as tile
    from concourse import mybir

    assert n_img % 2 == 0
    f32 = mybir.dt.float32
    bf16 = mybir.dt.bfloat16
    Alu = mybir.AluOpType
    npairs = n_img // 2

    _register_custom_ops()
    nc = bacc.Bacc("TRN2", target_bir_lowering=False, debug=False)

    io_kind = "Internal" if io_internal else "ExternalInput"
    ptin_d = nc.dram_tensor(
        "ptin", [130, npairs * NG, 2 * NSEC, S], bf16, kind=io_kind).ap()
    acc_d = nc.dram_tensor(
        "acc_out", [128, 8 * npairs], f32, kind="ExternalOutput").ap()

    with tile.TileContext(nc) as tc:
        with (
            tc.tile_pool(name="bufP", bufs=1) as poolP,
            tc.tile_pool(name="bufS", bufs=1) as poolS,
            tc.tile_pool(name="misc", bufs=1) as mpool,
        ):
            acc = mpool.tile([128, 8 * npairs], f32, tag="acc")

            for pair in range(npairs):
                gbase = pair * NG
                cbase = pair * 8
                for g in range(NG):
                    gi = gbase + g

                    def stage(off, nm, l1_col=None):
                        # load a copy into a bf16 stage tile and compute its
                        # [q|w] products; optionally the L1 accum (copy B)
                        t = poolS.tile([128, DW], bf16, tag="S", name=nm)
                        nc.sync.dma_start(
                            out=t[:, 0:WID].rearrange(
                                "p (f c) -> p f c", f=2 * NSEC, c=S),
                            in_=ptin_d[off:off + 128, gi, :, :])
                        if l1_col is not None:
                            nc.vector._custom_dve(
                                _OP_ABSD, out=t[:, WID:WID + HW_],
                                in0=t[:, 0:HW_], in1=t[:, HW_:WID],
                                accum_out=acc[:, l1_col:l1_col + 1])
                        nc.vector.scalar_tensor_tensor(
                            t[:, WID:WID + HW_], t[:, 0:HW_], 2.0,
                            t[:, HW_:WID], op0=Alu.mult, op1=Alu.mult)
                        nc.vector._custom_dve(
                            _OP_SQSUM, out=t[:, WID + HW_:DW],
                            in0=t[:, 0:HW_], in1=t[:, HW_:WID])
                        return t

                    SA = stage(0, "SA")
                    P = poolP.tile([128, DW], f32, tag="P", name="P")
                    nc.vector.tensor_copy(P[:, :], SA[:, :])
                    SB = stage(1, "SB", l1_col=cbase + g)
                    nc.vector.tensor_add(P[:, :], P[:, :], SB[:, :])
                    SC_ = stage(2, "SC")
                    nc.vector.tensor_add(P[:, :], P[:, :], SC_[:, :])
                    # P = Bv([p|t|q|w]) f32 (vertical 3-tap done)

                    # horizontal 3-tap, compact f32, one family-half at a
                    # time into the stage buffer (f32 instances)
                    Ppt = P[:, 0:WID].rearrange(
                        "p (f c) -> p f c", f=2 * NSEC, c=S)
                    Hpt = poolS.tile([128, 2 * CW], f32, tag="S", name="Hpt")
                    Hc = Hpt[:, :].rearrange(
                        "p (f c) -> p f c", f=2 * NSEC, c=W)
                    nc.vector.tensor_add(
                        Hc, Ppt[:, :, 0:W], Ppt[:, :, 1:W + 1])
                    nc.vector.tensor_add(Hc, Hc, Ppt[:, :, 2:W + 2])
                    # T = [A2=2XY | V=X^2+Y^2] -> P's dead [p|t] region
                    X = Hpt[:, 0:CW]
                    Y = Hpt[:, CW:2 * CW]
                    nc.vector.scalar_tensor_tensor(
                        P[:, 0:CW], X, 2.0, Y, op0=Alu.mult, op1=Alu.mult)
                    nc.vector._custom_dve(
                        _OP_SQSUM, out=P[:, CW:2 * CW], in0=X, in1=Y)
                    # H of the [q|w] half (P's qw region still live)
                    Pqw = P[:, WID:DW].rearrange(
                        "p (f c) -> p f c", f=2 * NSEC, c=S)
                    Hqw = poolS.tile([128, 2 * CW], f32, tag="S", name="Hqw")
                    Hq = Hqw[:, :].rearrange(
                        "p (f c) -> p f c", f=2 * NSEC, c=W)
                    nc.vector.tensor_add(
                        Hq, Pqw[:, :, 0:W], Pqw[:, :, 1:W + 1])
                    nc.vector.tensor_add(Hq, Hq, Pqw[:, :, 2:W + 2])

                    # fused rationals: R = (T+SC1)*(Hqw*SXS - T + SC2)
                    # in place -> [n1n2 | d1d2]
                    nc.vector._custom_dve(
                        _OP_SSIM_RAT, out=P[:, 0:2 * CW], in0=P[:, 0:2 * CW],
                        in1=Hqw[:, 0:2 * CW], s0=SC1, s1=SXS, imm2=SC2)
                    # reciprocal of d1d2 -> P's dead qw region, then fused
                    # clip-accumulate
                    nc.vector.reciprocal_approx_fast(
                        P[:, WID:WID + CW], P[:, CW:2 * CW])
                    nc.vector._custom_dve(
                        _OP_SSIM_FINAL, out=P[:, WID + CW:WID + 2 * CW],
                        in0=P[:, 0:CW], in1=P[:, WID:WID + CW],
                        s0=1.0, s1=-1.0, imm2=0.5,
                        accum_out=acc[:, cbase + 4 + g: cbase + 5 + g])

            nc.sync.dma_start(out=acc_d[:, :], in_=acc[:, :])

    nc.compile()
    return nc


_CACHE = {}


def _get_program(n_img, H, W):
    key = (n_img, H, W)
    if key not in _CACHE:
        _CACHE[key] = build_program(n_img, H, W)
    return _CACHE[key]


def make_bmats(H):
    """Compat stub for older harnesses (no matmuls in this kernel)."""
    return np.zeros((1, 1), dtype=np.float32)


def _pack_inputs(pred, target):
    """pred/target [n_img, H, W] -> packed [130, npairs*NG, 2*NSEC, S] bf16."""
    import ml_dtypes

    n_img, H, W = pred.shape
    assert n_img % 2 == 0
    npairs = n_img // 2
    out = np.zeros((130, npairs * NG, 2 * NSEC, S), dtype=ml_dtypes.bfloat16)
    pad_h = BLK * (NS - 1) + 130
    # dram j, stripe s -> padded row index 128*s + j (j=0 -> image row -1)
    J = (BLK * np.arange(NS)[None, :] + np.arange(130)[:, None])  # [130, NS]
    for pair in range(npairs):
        fields = (pred[2 * pair], pred[2 * pair + 1],
                  target[2 * pair], target[2 * pair + 1])
        for half in range(2):  # 0: p, 1: t
            for img in range(2):
                Pimg = np.zeros((pad_h, W), dtype=np.float32)
                Pimg[1:H + 1] = fields[2 * half + img]
                R = Pimg[J]  # [130, NS, W]
                for g in range(NG):
                    for s in range(KG):
                        out[:, pair * NG + g, half * NSEC + 2 * s + img,
                            1:W + 1] = R[:, g * KG + s].astype(
                                ml_dtypes.bfloat16)
    return out


LAST_RESULTS = None


def kernel(pred, target):
    from concourse.bass_utils import run_bass_kernel_spmd

    global LAST_RESULTS

    pred = np.asarray(pred, dtype=np.float32).reshape(16, IMG_H, IMG_W)
    target = np.asarray(target, dtype=np.float32).reshape(16, IMG_H, IMG_W)

    nc = _get_program(N_IMG_PER_CORE, IMG_H, IMG_W)

    in_maps = []
    for c in range(N_CORES):
        sl = slice(c * N_IMG_PER_CORE, (c + 1) * N_IMG_PER_CORE)
        in_maps.append({"ptin": _pack_inputs(pred[sl], target[sl])})

    res = run_bass_kernel_spmd(nc, in_maps, list(range(N_CORES)))
    LAST_RESULTS = res
    npairs = N_IMG_PER_CORE // 2
    ssim_sum = 0.0
    l1_sum = 0.0
    for r in res.results:
        acc = r["acc_out"]
        for p in range(npairs):
            b = 8 * p
            l1_sum += float(acc[:, b:b + NG].sum(dtype=np.float64))
            ssim_sum += float(acc[:, b + 4:b + 4 + NG].sum(dtype=np.float64))
    n = 16.0 * IMG_H * IMG_W
    loss = ALPHA * (ssim_sum / n) + BETA * (l1_sum / n)
    return np.float32(loss)
